# revision 12
# baseline (speedup 1.0000x reference)
"""Trainium2 Bass kernel for Exphormer sparse attention (GNN message passing).

Strategy:
  - Nodes are range-partitioned across the 8 cores by destination node id.
    Edges are sorted by dst on the host and routed to the core that owns
    their destination, so each core's local segment-sums are complete: no
    cross-core reduction is needed.
  - Each core builds the full K|V node table (bf16, interleaved 512B rows) in
    DRAM and its local Q slice in SBUF via matmuls, then processes edges in
    128-edge tiles grouped by 128-node dst blocks:
      * KV[src] rows arrive via a 128-row indirect-DMA gather per tile
      * Q[dst] is expanded from the SBUF-resident block window by a one-hot
        matmul (no gather needed: all dsts of a block live in one window)
      * T = K[src] * Q[dst]  (DVE); per-head weighted dots via PE:
        transpose(T), then T^T @ Wblk with Wblk = blockdiag(We/4 | be/4)
      * score = exp(clip(dw*a + db)); msg = V[src] * score
      * segment-sum via one-hot matmul accumulated in PSUM per dst block
  - Host work is index-only (sort, partition, pad) plus dtype/layout prep.
"""

import math
import os

import numpy as np
import ml_dtypes
import orjson

import concourse.bass as bass
import concourse.tile as tile
import concourse.mybir as mybir
from concourse.bass_utils import run_bass_kernel_spmd
from concourse.masks import make_identity

# ---------------------------------------------------------------------------
# Workaround: this walrus build supports only ONE sync-wait per instruction on
# core_v3, but Tile emits instructions with several.  Rewrite the BIR JSON
# right before compilation: peel surplus waits onto standalone EventSemaphore
# instructions inserted before the offender on the same engine (sequencers
# run in order, so sequential waiting is equivalent).
# ---------------------------------------------------------------------------
_WAIT_LIMIT = 1
_fix_installed = False


def _split_waits(bir_json: bytes, limit: int = _WAIT_LIMIT) -> bytes:
    d = orjson.loads(bir_json)
    ctr = 0
    changed = False
    for fn in d.get("functions", []):
        for blk in fn.get("blocks", []):
            out = []
            for inst in blk.get("instructions", []):
                si = inst.get("sync_info")
                ow = (si or {}).get("on_wait") or []
                if si is not None and len(ow) > limit and "engine" in inst:
                    keep = ow[-limit:]
                    for w in ow[:-limit]:
                        ctr += 1
                        out.append({
                            "debug": inst.get("debug", 0),
                            "engine": inst["engine"],
                            "ins": [],
                            "outs": [],
                            "name": f"antsw-{ctr}-{inst['name']}",
                            "opcode": "EventSemaphore",
                            "sync_info": {"on_update": [], "on_wait": [w]},
                        })
                    si["on_wait"] = keep
                    changed = True
                out.append(inst)
            blk["instructions"] = out
    if not changed:
        return bir_json
    return orjson.dumps(d)


def _install_wait_fix():
    global _fix_installed
    if _fix_installed:
        return
    _fix_installed = True
    import concourse.bass_utils as bu
    import concourse.bass2jax as b2j

    orig = bu.compile_bir_kernel

    def wrapped(bir_json, tmpdir, neff_name="file.neff"):
        return orig(_split_waits(bytes(bir_json)), tmpdir, neff_name=neff_name)

    bu.compile_bir_kernel = wrapped
    b2j.compile_bir_kernel = wrapped


P = 128
F32 = mybir.dt.float32
BF16 = mybir.dt.bfloat16
I32 = mybir.dt.int32
AX = mybir.AluOpType
AF = mybir.ActivationFunctionType

LAST_RESULTS = None  # test harness reads exec_time_ns from here


SPLIT = 32768  # int16 gather-index limit: src < SPLIT uses the lo table view


class Cfg:
    def __init__(self, n_nodes=50000, n_cores=8):
        self.n_nodes = n_nodes
        self.n_cores = n_cores
        self.npc = n_nodes // n_cores            # nodes per core
        self.nb = math.ceil(self.npc / P)        # dst blocks per core
        self.qrows = self.nb * P                 # padded local Q rows
        need = max(n_nodes, (n_cores - 1) * self.npc + self.qrows)
        self.npad = math.ceil(need / P) * P
        # filled by host_prep (shared across cores so the BIR is SPMD):
        self.lo_t = None   # per-block lo-half tile count
        self.hi_t = None   # per-block hi-half tile count
        self.bt = None     # per-block total tiles
        self.bo = None     # per-block tile offset
        self.tt = None     # total tiles per core


def build_nc(cfg: Cfg):
    from concourse import library_config
    from concourse.library_overlay import lower_extended_insts

    nc = bass.Bass("TRN2", target_bir_lowering=False, num_devices=cfg.n_cores)

    I16 = mybir.dt.int16
    XT = nc.dram_tensor("xt", [P, cfg.npad], BF16, kind="ExternalInput").ap()
    XTQ = nc.dram_tensor("xtq", [P, cfg.qrows], BF16, kind="ExternalInput").ap()
    WKV = nc.dram_tensor("wkv", [P, 256], F32, kind="ExternalInput").ap()
    BKV = nc.dram_tensor("bkv", [1, 256], F32, kind="ExternalInput").ap()
    WQ = nc.dram_tensor("wq", [P, P], F32, kind="ExternalInput").ap()
    BQ = nc.dram_tensor("bq", [1, P], F32, kind="ExternalInput").ap()
    WE = nc.dram_tensor("we", [1, P], F32, kind="ExternalInput").ap()
    BE = nc.dram_tensor("be", [1, P], F32, kind="ExternalInput").ap()
    IDX = nc.dram_tensor("idxw", [P, cfg.tt * 8], I16, kind="ExternalInput").ap()
    IDENT = nc.dram_tensor("identm", [P, P], BF16, kind="ExternalInput").ap()
    IOTAR = nc.dram_tensor("iotar", [P, P], BF16, kind="ExternalInput").ap()
    IOTAC = nc.dram_tensor("iotac", [P, 1], F32, kind="ExternalInput").ap()
    AT = nc.dram_tensor("at", [P, cfg.tt], F32, kind="ExternalInput").ap()
    IBT = nc.dram_tensor("ibt", [P, cfg.tt], F32, kind="ExternalInput").ap()
    IBR = nc.dram_tensor("ibr", [1, cfg.tt * P], BF16, kind="ExternalInput").ap()
    HOUT = nc.dram_tensor("hout", [cfg.qrows, P], F32, kind="ExternalOutput").ap()

    KVT = nc.dram_tensor("kvt", [cfg.npad, 256], BF16).ap()

    n_kv_tiles = cfg.npad // P
    n_q_tiles = cfg.qrows // P
    SLAB = 8  # node tiles per x-slab load

    with tile.TileContext(nc) as tc:
        with (
            tc.tile_pool(name="const", bufs=1) as cpool,
            tc.tile_pool(name="meta", bufs=1) as mpool,
            tc.tile_pool(name="qres", bufs=1) as qpool,
        ):
            # ---- constants ----
            ident = cpool.tile([P, P], BF16)
            nc.sync.dma_start(out=ident[:], in_=IDENT[:])
            ones_row = cpool.tile([1, P], BF16)
            nc.gpsimd.memset(ones_row[:], 1.0)
            iota_f = cpool.tile([P, P], BF16)
            nc.sync.dma_start(out=iota_f[:], in_=IOTAR[:])
            iotac_f = cpool.tile([P, 1], F32)
            nc.sync.dma_start(out=iotac_f[:], in_=IOTAC[:])

            wkv_f = cpool.tile([P, 256], F32)
            nc.sync.dma_start(out=wkv_f[:], in_=WKV[:])
            wkv_sb = cpool.tile([P, 256], BF16)
            nc.vector.tensor_copy(out=wkv_sb[:], in_=wkv_f[:])
            bkv_f = cpool.tile([1, 256], F32)
            nc.sync.dma_start(out=bkv_f[:], in_=BKV[:])
            bkv_sb = cpool.tile([1, 256], BF16)
            nc.vector.tensor_copy(out=bkv_sb[:], in_=bkv_f[:])
            wq_f = cpool.tile([P, P], F32)
            nc.sync.dma_start(out=wq_f[:], in_=WQ[:])
            wq_sb = cpool.tile([P, P], BF16)
            nc.vector.tensor_copy(out=wq_sb[:], in_=wq_f[:])
            bq_f = cpool.tile([1, P], F32)
            nc.sync.dma_start(out=bq_f[:], in_=BQ[:])
            bq_sb = cpool.tile([1, P], BF16)
            nc.vector.tensor_copy(out=bq_sb[:], in_=bq_f[:])

            # Wblk = block-diag(We/4 | be/4): [128, 16]
            we_f = cpool.tile([1, P], F32)
            nc.sync.dma_start(out=we_f[:], in_=WE[:])
            be_f = cpool.tile([1, P], F32)
            nc.sync.dma_start(out=be_f[:], in_=BE[:])
            we4 = cpool.tile([1, P], F32)
            nc.vector.tensor_scalar(out=we4[:], in0=we_f[:], scalar1=0.25,
                                    scalar2=None, op0=AX.mult)
            be4 = cpool.tile([1, P], F32)
            nc.vector.tensor_scalar(out=be4[:], in0=be_f[:], scalar1=0.25,
                                    scalar2=None, op0=AX.mult)
            wblk_f = cpool.tile([P, 16], F32)
            nc.gpsimd.memset(wblk_f[:], 0.0)
            for h in range(8):
                nc.sync.dma_start(out=wblk_f[16 * h:16 * h + 16, h:h + 1],
                                  in_=we4[0:1, 16 * h:16 * h + 16])
                nc.sync.dma_start(out=wblk_f[16 * h:16 * h + 16, 8 + h:8 + h + 1],
                                  in_=be4[0:1, 16 * h:16 * h + 16])
            wblk = cpool.tile([P, 16], BF16)
            nc.vector.tensor_copy(out=wblk[:], in_=wblk_f[:])

            # ---- resident edge metadata ----
            idx_sb = mpool.tile([P, cfg.tt * 8], I16)
            nc.sync.dma_start(out=idx_sb[:], in_=IDX[:])
            a_sb = mpool.tile([P, cfg.tt], F32)
            nc.sync.dma_start(out=a_sb[:], in_=AT[:])
            ib_sb = mpool.tile([P, cfg.tt], F32)
            nc.sync.dma_start(out=ib_sb[:], in_=IBT[:])

            # ---- local Q, SBUF-resident [128, nb*128] (node b*128+p at
            # partition p, cols b*128..) ----
            qres = qpool.tile([P, cfg.qrows], BF16)

            with (
                tc.tile_pool(name="bld", bufs=3) as bpool,
                tc.tile_pool(name="bldp", bufs=2, space="PSUM") as bppool,
            ):
                for j in range(n_q_tiles):
                    if j % SLAB == 0:
                        xq = bpool.tile([P, SLAB * P], BF16, tag="xq")
                        w = min(SLAB * P, cfg.qrows - j * P)
                        nc.sync.dma_start(out=xq[:, :w], in_=XTQ[:, j * P:j * P + w])
                    lhs = xq[:, (j % SLAB) * P:(j % SLAB + 1) * P]
                    qp = bppool.tile([P, P], F32, tag="qp")
                    nc.tensor.matmul(out=qp[:], lhsT=lhs, rhs=wq_sb[:],
                                     start=True, stop=False)
                    nc.tensor.matmul(out=qp[:], lhsT=ones_row[:1, :], rhs=bq_sb[:1, :],
                                     start=False, stop=True)
                    if j % 2 == 0:
                        nc.scalar.activation(out=qres[:, j * P:(j + 1) * P],
                                             in_=qp[:], func=AF.Copy)
                    else:
                        nc.vector.tensor_copy(out=qres[:, j * P:(j + 1) * P],
                                              in_=qp[:])

                for i in range(n_kv_tiles):
                    if i % SLAB == 0:
                        xs = bpool.tile([P, SLAB * P], BF16, tag="xs")
                        w = min(SLAB * P, cfg.npad - i * P)
                        nc.sync.dma_start(out=xs[:, :w], in_=XT[:, i * P:i * P + w])
                    lhs = xs[:, (i % SLAB) * P:(i % SLAB + 1) * P]
                    bp = bppool.tile([P, 256], F32, tag="bp")
                    nc.tensor.matmul(out=bp[:], lhsT=lhs, rhs=wkv_sb[:],
                                     start=True, stop=False)
                    nc.tensor.matmul(out=bp[:], lhsT=ones_row[:1, :], rhs=bkv_sb[:1, :],
                                     start=False, stop=True)
                    kvsb = bpool.tile([P, 256], BF16, tag="kvsb")
                    if i % 2 == 0:
                        nc.scalar.activation(out=kvsb[:], in_=bp[:], func=AF.Copy)
                    else:
                        nc.vector.tensor_copy(out=kvsb[:], in_=bp[:])
                    nc.sync.dma_start(out=KVT[i * P:(i + 1) * P, :], in_=kvsb[:])

            # ---- edge phase ----
            nc.gpsimd.load_library(library_config.mlp)
            btmax = max(cfg.bt)
            with (
                tc.tile_pool(name="kvg", bufs=2) as kvpool,
                tc.tile_pool(name="work", bufs=4) as wpool,
                tc.tile_pool(name="ibrp", bufs=2) as ibrpool,
                tc.tile_pool(name="qxp", bufs=3, space="PSUM") as qxpool,
                tc.tile_pool(name="ttp", bufs=3, space="PSUM") as ttpool,
                tc.tile_pool(name="accp", bufs=2, space="PSUM") as accpool,
            ):
                nireg = nc.alloc_register(mybir.EngineType.Pool, "nidx")
                for blk in range(cfg.nb):
                    t0 = cfg.bo[blk]
                    bt, lo_t, hi_t = cfg.bt[blk], cfg.lo_t[blk], cfg.hi_t[blk]
                    qwin = qres[:, blk * P:(blk + 1) * P]
                    ibb = ibrpool.tile([P, btmax * P], BF16, tag="ibb")
                    nc.sync.dma_start(
                        out=ibb[:, :bt * P],
                        in_=IBR[0:1, t0 * P:(t0 + bt) * P].to_broadcast(
                            (P, bt * P)))
                    kvg = kvpool.tile([P, btmax, 256], BF16, tag="kvg")
                    CH = 2  # tiles per gather call (256 idxs: HW-validated size)
                    for g0 in range(0, lo_t, CH):
                        gt = min(CH, lo_t - g0)
                        nc.gpsimd.reg_mov(nireg, gt * P)
                        nc.gpsimd.dma_gather(
                            out_ap=kvg[:, g0:g0 + gt, :], in_ap=KVT[:SPLIT, :],
                            idxs_ap=idx_sb[:, (t0 + g0) * 8:(t0 + g0 + gt) * 8],
                            num_idxs=gt * P, num_idxs_reg=nireg,
                            elem_size=256)
                    for g0 in range(0, hi_t, CH):
                        gt = min(CH, hi_t - g0)
                        nc.gpsimd.reg_mov(nireg, gt * P)
                        nc.gpsimd.dma_gather(
                            out_ap=kvg[:, lo_t + g0:lo_t + g0 + gt, :],
                            in_ap=KVT[SPLIT:, :],
                            idxs_ap=idx_sb[:, (t0 + lo_t + g0) * 8:(t0 + lo_t + g0 + gt) * 8],
                            num_idxs=gt * P, num_idxs_reg=nireg,
                            elem_size=256)
                    acc = accpool.tile([P, 136], F32, tag="acc")
                    for ti in range(bt):
                        T = t0 + ti
                        qx = qxpool.tile([P, 144], F32, tag="qx")
                        oh2 = wpool.tile([P, P], BF16, tag="oh2")
                        nc.vector.tensor_scalar(
                            out=oh2[:], in0=ibb[:, ti * P:(ti + 1) * P],
                            scalar1=iotac_f[:, 0:1], scalar2=None, op0=AX.is_equal)
                        nc.tensor.matmul(out=qx[:, 0:128], lhsT=oh2[:], rhs=qwin,
                                         start=True, stop=True)
                        ttile = wpool.tile([P, P], BF16, tag="ttile")
                        nc.vector.tensor_tensor(
                            out=ttile[:], in0=kvg[:, ti, 0:128], in1=qx[:, 0:128],
                            op=AX.mult)
                        ttp = ttpool.tile([P, P], BF16, tag="ttp")
                        nc.tensor.transpose(out=ttp[:], in_=ttile[:],
                                            identity=ident[:])
                        tts = wpool.tile([P, P], BF16, tag="tts")
                        nc.scalar.activation(out=tts[:], in_=ttp[:], func=AF.Copy)
                        nc.tensor.matmul(out=qx[:, 128:144], lhsT=tts[:],
                                         rhs=wblk[:], start=True, stop=True)
                        u = wpool.tile([P, 8], F32, tag="u")
                        nc.vector.tensor_scalar(
                            out=u[:], in0=qx[:, 128:136],
                            scalar1=a_sb[:, T:T + 1], scalar2=None, op0=AX.mult)
                        sc = wpool.tile([P, 8], F32, tag="sc")
                        nc.vector.tensor_tensor(out=sc[:], in0=u[:],
                                                in1=qx[:, 136:144], op=AX.add)
                        scc = wpool.tile([P, 8], F32, tag="scc")
                        nc.vector.tensor_scalar(out=scc[:], in0=sc[:],
                                                scalar1=5.0, scalar2=-5.0,
                                                op0=AX.min, op1=AX.max)
                        msgz = wpool.tile([P, 136], BF16, tag="msgz")
                        nc.scalar.activation(out=msgz[:, 128:136], in_=scc[:],
                                             func=AF.Exp)
                        nc.vector.tensor_tensor(
                            out=msgz[:, 0:128].rearrange("p (h d) -> p h d", d=16),
                            in0=kvg[:, ti, 128:256].rearrange("p (h d) -> p h d", d=16),
                            in1=msgz[:, 128:136][:, :, None].to_broadcast((P, 8, 16)),
                            op=AX.mult)
                        oh = wpool.tile([P, P], BF16, tag="oh")
                        nc.vector.tensor_scalar(
                            out=oh[:], in0=iota_f[:],
                            scalar1=ib_sb[:, T:T + 1], scalar2=None,
                            op0=AX.is_equal)
                        nc.tensor.matmul(out=acc[:], lhsT=oh[:], rhs=msgz[:],
                                         start=(ti == 0),
                                         stop=(ti == bt - 1))
                    zp = wpool.tile([P, 8], F32, tag="zp")
                    nc.vector.tensor_scalar(out=zp[:], in0=acc[:, 128:136],
                                            scalar1=1e-6, scalar2=None, op0=AX.add)
                    rt = wpool.tile([P, 8], F32, tag="rt")
                    nc.vector.reciprocal(out=rt[:], in_=zp[:])
                    ho = wpool.tile([P, P], F32, tag="ho")
                    nc.vector.tensor_tensor(
                        out=ho[:].rearrange("p (h d) -> p h d", d=16),
                        in0=acc[:, 0:128].rearrange("p (h d) -> p h d", d=16),
                        in1=rt[:][:, :, None].to_broadcast((P, 8, 16)),
                        op=AX.mult)
                    nc.sync.dma_start(out=HOUT[blk * P:(blk + 1) * P, :], in_=ho[:])
    lower_extended_insts(nc)
    return nc


def host_prep(x, eidx, eattr, cfg: Cfg):
    """Index-only edge prep + dtype/layout prep of inputs. Returns in_maps.

    Edges are routed to the core owning their dst, grouped into 128-node dst
    blocks, and within each block sorted lo-half-first by src (< SPLIT) so
    each half is one contiguous dma_gather with int16 indices relative to
    its table-half view. Tile counts per (block, half) are maxed over cores
    so the BIR stays SPMD."""
    n, _ = x.shape
    src = np.asarray(eidx[0], dtype=np.int64)
    dst = np.asarray(eidx[1], dtype=np.int64)
    a = np.asarray(eattr, dtype=np.float32).reshape(-1)

    order = np.argsort(dst, kind="stable")
    src_s, dst_s, a_s = src[order], dst[order], a[order]
    core_of = np.minimum(dst_s // cfg.npc, cfg.n_cores - 1)

    per_core = []
    lo_cnt = np.zeros((cfg.n_cores, cfg.nb), dtype=np.int64)
    hi_cnt = np.zeros((cfg.n_cores, cfg.nb), dtype=np.int64)
    for c in range(cfg.n_cores):
        m = core_of == c
        s, d, av = src_s[m], dst_s[m] - c * cfg.npc, a_s[m]
        blk = d >> 7
        hi = (s >= SPLIT).astype(np.int64)
        # stable sort by (blk, hi): lo edges of each block first
        o = np.lexsort((hi, blk))
        s, d, av, blk, hi = s[o], d[o], av[o], blk[o], hi[o]
        np.add.at(lo_cnt[c], blk[hi == 0], 1)
        np.add.at(hi_cnt[c], blk[hi == 1], 1)
        per_core.append((s, d, av, blk, hi))

    cfg.lo_t = [int(math.ceil(lo_cnt[:, b].max() / P)) for b in range(cfg.nb)]
    cfg.hi_t = [int(math.ceil(hi_cnt[:, b].max() / P)) for b in range(cfg.nb)]
    cfg.bt = [l + h for l, h in zip(cfg.lo_t, cfg.hi_t)]
    cfg.bo = np.concatenate([[0], np.cumsum(cfg.bt)[:-1]]).tolist()
    cfg.tt = int(sum(cfg.bt))

    xt_pad = np.zeros((cfg.npad, x.shape[1]), dtype=np.float32)
    xt_pad[:n] = x
    xt_bf = np.ascontiguousarray(xt_pad.T).astype(ml_dtypes.bfloat16)

    bo = np.asarray(cfg.bo, dtype=np.int64)
    lo_t = np.asarray(cfg.lo_t, dtype=np.int64)

    in_maps = []
    for c in range(cfg.n_cores):
        s, d, av, blk, hi = per_core[c]
        # slot within block: lo edges pack from block start, hi edges pack
        # from the (shared) lo_t tile boundary
        lo_starts = np.zeros(cfg.nb, dtype=np.int64)
        lo_starts[1:] = np.cumsum(lo_cnt[c])[:-1]
        hi_starts = np.zeros(cfg.nb, dtype=np.int64)
        hi_starts[1:] = np.cumsum(hi_cnt[c])[:-1]
        pos_all = np.arange(len(s))
        lo_m = hi == 0
        pos = np.where(lo_m, pos_all - lo_starts[blk] - hi_starts[blk],
                       pos_all - lo_starts[blk] - hi_starts[blk] - lo_cnt[c][blk])
        base = np.where(lo_m, bo[blk] * P, (bo[blk] + lo_t[blk]) * P)
        slot = base + pos

        tot = cfg.tt * P
        IDXa = np.zeros(tot, dtype=np.int16)
        Aa = np.zeros(tot, dtype=np.float32)
        IBa = np.full(tot, 200.0, dtype=np.float32)
        IDXa[slot] = np.where(lo_m, s, s - SPLIT).astype(np.int16)
        Aa[slot] = av
        IBa[slot] = (d - blk * P).astype(np.float32)

        def tmat(arr):
            return np.ascontiguousarray(arr.reshape(cfg.tt, P).T)

        idxw = np.ascontiguousarray(IDXa.reshape(cfg.tt * 8, 16).T)
        idxw = np.tile(idxw, (8, 1))  # replicate across 16-partition groups

        ident_np = np.eye(P, dtype=ml_dtypes.bfloat16)
        iotar_np = np.tile(np.arange(P, dtype=np.float32), (P, 1)).astype(ml_dtypes.bfloat16)
        iotac_np = np.arange(P, dtype=np.float32).reshape(P, 1)
        in_maps.append({
            "identm": ident_np, "iotar": iotar_np, "iotac": iotac_np,
            "xt": xt_bf,
            "xtq": np.ascontiguousarray(
                xt_bf[:, c * cfg.npc:c * cfg.npc + cfg.qrows]),
            "idxw": idxw,
            "at": tmat(Aa),
            "ibt": tmat(IBa),
            "ibr": np.ascontiguousarray(
                IBa.reshape(1, cfg.tt * P)).astype(ml_dtypes.bfloat16),
        })
    return in_maps


def _weight_maps(Wq, bq, Wk, bk, We, be, Wv, bv):
    wkv = np.concatenate([Wk, Wv], axis=1).astype(np.float32)
    return {
        "wkv": np.ascontiguousarray(wkv),
        "bkv": np.concatenate([bk, bv]).astype(np.float32).reshape(1, 256),
        "wq": np.ascontiguousarray(Wq.astype(np.float32)),
        "bq": bq.astype(np.float32).reshape(1, -1),
        "we": We.astype(np.float32).reshape(1, -1),
        "be": be.astype(np.float32).reshape(1, -1),
    }


def kernel(**inputs):
    global LAST_RESULTS
    _install_wait_fix()
    x = np.asarray(inputs["x"], dtype=np.float32)
    cfg = Cfg(n_nodes=x.shape[0])
    in_maps = host_prep(x, inputs["expander_edge_index"],
                        inputs["expander_edge_attr"], cfg)
    wm = _weight_maps(np.asarray(inputs["Wq"]), np.asarray(inputs["bq"]),
                      np.asarray(inputs["Wk"]), np.asarray(inputs["bk"]),
                      np.asarray(inputs["We"]), np.asarray(inputs["be"]),
                      np.asarray(inputs["Wv"]), np.asarray(inputs["bv"]))
    for im in in_maps:
        im.update(wm)

    nc = build_nc(cfg)
    trace = os.environ.get("KERNEL_TRACE", "0") == "1"
    res = run_bass_kernel_spmd(nc, in_maps, list(range(cfg.n_cores)), trace=trace)
    LAST_RESULTS = res

    out = np.empty((x.shape[0], x.shape[1]), dtype=np.float32)
    for c in range(cfg.n_cores):
        out[c * cfg.npc:(c + 1) * cfg.npc] = res.results[c]["hout"][:cfg.npc]
    return out



# revision 13
# speedup vs baseline: 1.3099x; 1.3099x over previous
"""Trainium2 Bass kernel for Exphormer sparse attention (GNN message passing).

Strategy:
  - Nodes are range-partitioned across the 8 cores by destination node id.
    Edges are sorted by dst on the host and routed to the core that owns
    their destination, so each core's local segment-sums are complete: no
    cross-core reduction is needed.
  - Each core builds the full K|V node table (bf16, interleaved 512B rows) in
    DRAM and its local Q slice in SBUF via matmuls, then processes edges in
    128-edge tiles grouped by 128-node dst blocks:
      * KV[src] rows arrive via a 128-row indirect-DMA gather per tile
      * Q[dst] is expanded from the SBUF-resident block window by a one-hot
        matmul (no gather needed: all dsts of a block live in one window)
      * T = K[src] * Q[dst]  (DVE); per-head weighted dots via PE:
        transpose(T), then T^T @ Wblk with Wblk = blockdiag(We/4 | be/4)
      * score = exp(clip(dw*a + db)); msg = V[src] * score
      * segment-sum via one-hot matmul accumulated in PSUM per dst block
  - Host work is index-only (sort, partition, pad) plus dtype/layout prep.
"""

import math
import os

import numpy as np
import ml_dtypes
import orjson

import concourse.bass as bass
import concourse.tile as tile
import concourse.mybir as mybir
from concourse.bass_utils import run_bass_kernel_spmd
from concourse.masks import make_identity

# ---------------------------------------------------------------------------
# Workaround: this walrus build supports only ONE sync-wait per instruction on
# core_v3, but Tile emits instructions with several.  Rewrite the BIR JSON
# right before compilation: peel surplus waits onto standalone EventSemaphore
# instructions inserted before the offender on the same engine (sequencers
# run in order, so sequential waiting is equivalent).
# ---------------------------------------------------------------------------
_WAIT_LIMIT = 1
_fix_installed = False


def _split_waits(bir_json: bytes, limit: int = _WAIT_LIMIT) -> bytes:
    d = orjson.loads(bir_json)
    ctr = 0
    changed = False
    for fn in d.get("functions", []):
        for blk in fn.get("blocks", []):
            out = []
            for inst in blk.get("instructions", []):
                si = inst.get("sync_info")
                ow = (si or {}).get("on_wait") or []
                if si is not None and len(ow) > limit and "engine" in inst:
                    keep = ow[-limit:]
                    for w in ow[:-limit]:
                        ctr += 1
                        out.append({
                            "debug": inst.get("debug", 0),
                            "engine": inst["engine"],
                            "ins": [],
                            "outs": [],
                            "name": f"antsw-{ctr}-{inst['name']}",
                            "opcode": "EventSemaphore",
                            "sync_info": {"on_update": [], "on_wait": [w]},
                        })
                    si["on_wait"] = keep
                    changed = True
                out.append(inst)
            blk["instructions"] = out
    if not changed:
        return bir_json
    return orjson.dumps(d)


def _install_wait_fix():
    global _fix_installed
    if _fix_installed:
        return
    _fix_installed = True
    import concourse.bass_utils as bu
    import concourse.bass2jax as b2j

    orig = bu.compile_bir_kernel

    def wrapped(bir_json, tmpdir, neff_name="file.neff"):
        return orig(_split_waits(bytes(bir_json)), tmpdir, neff_name=neff_name)

    bu.compile_bir_kernel = wrapped
    b2j.compile_bir_kernel = wrapped


P = 128
F32 = mybir.dt.float32
BF16 = mybir.dt.bfloat16
I32 = mybir.dt.int32
AX = mybir.AluOpType
AF = mybir.ActivationFunctionType

LAST_RESULTS = None  # test harness reads exec_time_ns from here


class Cfg:
    def __init__(self, n_nodes=50000, n_cores=8, b_tiles=18):
        self.n_nodes = n_nodes
        self.n_cores = n_cores
        self.npc = n_nodes // n_cores            # nodes per core
        self.nb = math.ceil(self.npc / P)        # dst blocks per core
        self.qrows = self.nb * P                 # padded local Q rows
        need = max(n_nodes, (n_cores - 1) * self.npc + self.qrows)
        self.npad = math.ceil(need / P) * P
        self.b_tiles = b_tiles                   # 128-edge tiles per dst block
        self.tt = self.nb * b_tiles              # total edge tiles per core


def build_nc(cfg: Cfg):
    mq = int(os.environ.get("KERNEL_MQ", "1"))
    kvgbufs = int(os.environ.get("KERNEL_KVGBUFS", "8"))
    nc = bass.Bass("TRN2", target_bir_lowering=False, num_devices=cfg.n_cores,
                   num_swdge_queues=mq)

    XT = nc.dram_tensor("xt", [P, cfg.npad], BF16, kind="ExternalInput").ap()
    XTQ = nc.dram_tensor("xtq", [P, cfg.qrows], BF16, kind="ExternalInput").ap()
    WKV = nc.dram_tensor("wkv", [P, 256], F32, kind="ExternalInput").ap()
    BKV = nc.dram_tensor("bkv", [1, 256], F32, kind="ExternalInput").ap()
    WQ = nc.dram_tensor("wq", [P, P], F32, kind="ExternalInput").ap()
    BQ = nc.dram_tensor("bq", [1, P], F32, kind="ExternalInput").ap()
    WE = nc.dram_tensor("we", [1, P], F32, kind="ExternalInput").ap()
    BE = nc.dram_tensor("be", [1, P], F32, kind="ExternalInput").ap()
    SRC = nc.dram_tensor("srct", [P, cfg.tt], I32, kind="ExternalInput").ap()
    AT = nc.dram_tensor("at", [P, cfg.tt], F32, kind="ExternalInput").ap()
    IBT = nc.dram_tensor("ibt", [P, cfg.tt], F32, kind="ExternalInput").ap()
    IBR = nc.dram_tensor("ibr", [1, cfg.tt * P], F32, kind="ExternalInput").ap()
    HOUT = nc.dram_tensor("hout", [cfg.qrows, P], F32, kind="ExternalOutput").ap()

    KVT = nc.dram_tensor("kvt", [cfg.npad, 256], BF16).ap()

    n_kv_tiles = cfg.npad // P
    n_q_tiles = cfg.qrows // P
    SLAB = 8  # node tiles per x-slab load

    with tile.TileContext(nc) as tc:
        with (
            tc.tile_pool(name="const", bufs=1) as cpool,
            tc.tile_pool(name="meta", bufs=1) as mpool,
            tc.tile_pool(name="qres", bufs=1) as qpool,
        ):
            # ---- constants ----
            ident = cpool.tile([P, P], BF16)
            make_identity(nc, ident[:])
            ones_row = cpool.tile([1, P], BF16)
            nc.gpsimd.memset(ones_row[:], 1.0)
            iota_i = cpool.tile([P, P], I32)
            nc.gpsimd.iota(iota_i[:], pattern=[[1, P]], base=0, channel_multiplier=0)
            iota_f = cpool.tile([P, P], F32)
            nc.vector.tensor_copy(out=iota_f[:], in_=iota_i[:])
            iotac_i = cpool.tile([P, 1], I32)
            nc.gpsimd.iota(iotac_i[:], pattern=[[0, 1]], base=0, channel_multiplier=1)
            iotac_f = cpool.tile([P, 1], F32)
            nc.vector.tensor_copy(out=iotac_f[:], in_=iotac_i[:])

            wkv_f = cpool.tile([P, 256], F32)
            nc.sync.dma_start(out=wkv_f[:], in_=WKV[:])
            wkv_sb = cpool.tile([P, 256], BF16)
            nc.vector.tensor_copy(out=wkv_sb[:], in_=wkv_f[:])
            bkv_f = cpool.tile([1, 256], F32)
            nc.sync.dma_start(out=bkv_f[:], in_=BKV[:])
            bkv_sb = cpool.tile([1, 256], BF16)
            nc.vector.tensor_copy(out=bkv_sb[:], in_=bkv_f[:])
            wq_f = cpool.tile([P, P], F32)
            nc.sync.dma_start(out=wq_f[:], in_=WQ[:])
            wq_sb = cpool.tile([P, P], BF16)
            nc.vector.tensor_copy(out=wq_sb[:], in_=wq_f[:])
            bq_f = cpool.tile([1, P], F32)
            nc.sync.dma_start(out=bq_f[:], in_=BQ[:])
            bq_sb = cpool.tile([1, P], BF16)
            nc.vector.tensor_copy(out=bq_sb[:], in_=bq_f[:])

            # Wblk = block-diag(We/4 | be/4): [128, 16]
            we_f = cpool.tile([1, P], F32)
            nc.sync.dma_start(out=we_f[:], in_=WE[:])
            be_f = cpool.tile([1, P], F32)
            nc.sync.dma_start(out=be_f[:], in_=BE[:])
            we4 = cpool.tile([1, P], F32)
            nc.vector.tensor_scalar(out=we4[:], in0=we_f[:], scalar1=0.25,
                                    scalar2=None, op0=AX.mult)
            be4 = cpool.tile([1, P], F32)
            nc.vector.tensor_scalar(out=be4[:], in0=be_f[:], scalar1=0.25,
                                    scalar2=None, op0=AX.mult)
            wblk_f = cpool.tile([P, 16], F32)
            nc.gpsimd.memset(wblk_f[:], 0.0)
            for h in range(8):
                nc.sync.dma_start(out=wblk_f[16 * h:16 * h + 16, h:h + 1],
                                  in_=we4[0:1, 16 * h:16 * h + 16])
                nc.sync.dma_start(out=wblk_f[16 * h:16 * h + 16, 8 + h:8 + h + 1],
                                  in_=be4[0:1, 16 * h:16 * h + 16])
            wblk = cpool.tile([P, 16], BF16)
            nc.vector.tensor_copy(out=wblk[:], in_=wblk_f[:])

            # ---- resident edge metadata ----
            src_sb = mpool.tile([P, cfg.tt], I32)
            nc.sync.dma_start(out=src_sb[:], in_=SRC[:])
            a_sb = mpool.tile([P, cfg.tt], F32)
            nc.sync.dma_start(out=a_sb[:], in_=AT[:])
            ib_sb = mpool.tile([P, cfg.tt], F32)
            nc.sync.dma_start(out=ib_sb[:], in_=IBT[:])

            # ---- local Q, SBUF-resident [128, nb*128] (node b*128+p at
            # partition p, cols b*128..) ----
            qres = qpool.tile([P, cfg.qrows], BF16)

            with (
                tc.tile_pool(name="bld", bufs=3) as bpool,
                tc.tile_pool(name="bldp", bufs=2, space="PSUM") as bppool,
            ):
                for j in range(n_q_tiles):
                    if j % SLAB == 0:
                        xq = bpool.tile([P, SLAB * P], BF16, tag="xq")
                        w = min(SLAB * P, cfg.qrows - j * P)
                        nc.sync.dma_start(out=xq[:, :w], in_=XTQ[:, j * P:j * P + w])
                    lhs = xq[:, (j % SLAB) * P:(j % SLAB + 1) * P]
                    qp = bppool.tile([P, P], F32, tag="qp")
                    nc.tensor.matmul(out=qp[:], lhsT=lhs, rhs=wq_sb[:],
                                     start=True, stop=False)
                    nc.tensor.matmul(out=qp[:], lhsT=ones_row[:1, :], rhs=bq_sb[:1, :],
                                     start=False, stop=True)
                    if j % 2 == 0:
                        nc.scalar.activation(out=qres[:, j * P:(j + 1) * P],
                                             in_=qp[:], func=AF.Copy)
                    else:
                        nc.vector.tensor_copy(out=qres[:, j * P:(j + 1) * P],
                                              in_=qp[:])

                for i in range(n_kv_tiles):
                    if i % SLAB == 0:
                        xs = bpool.tile([P, SLAB * P], BF16, tag="xs")
                        w = min(SLAB * P, cfg.npad - i * P)
                        nc.sync.dma_start(out=xs[:, :w], in_=XT[:, i * P:i * P + w])
                    lhs = xs[:, (i % SLAB) * P:(i % SLAB + 1) * P]
                    bp = bppool.tile([P, 256], F32, tag="bp")
                    nc.tensor.matmul(out=bp[:], lhsT=lhs, rhs=wkv_sb[:],
                                     start=True, stop=False)
                    nc.tensor.matmul(out=bp[:], lhsT=ones_row[:1, :], rhs=bkv_sb[:1, :],
                                     start=False, stop=True)
                    kvsb = bpool.tile([P, 256], BF16, tag="kvsb")
                    if i % 2 == 0:
                        nc.scalar.activation(out=kvsb[:], in_=bp[:], func=AF.Copy)
                    else:
                        nc.vector.tensor_copy(out=kvsb[:], in_=bp[:])
                    nc.sync.dma_start(out=KVT[i * P:(i + 1) * P, :], in_=kvsb[:])

            # ---- edge phase ----
            with (
                tc.tile_pool(name="kvg", bufs=kvgbufs) as kvpool,
                tc.tile_pool(name="work", bufs=4) as wpool,
                tc.tile_pool(name="ibrp", bufs=2) as ibrpool,
                tc.tile_pool(name="qxp", bufs=3, space="PSUM") as qxpool,
                tc.tile_pool(name="ttp", bufs=3, space="PSUM") as ttpool,
                tc.tile_pool(name="accp", bufs=2, space="PSUM") as accpool,
            ):
                for blk in range(cfg.nb):
                    t0 = blk * cfg.b_tiles
                    qwin = qres[:, blk * P:(blk + 1) * P]
                    ibb = ibrpool.tile([P, cfg.b_tiles * P], F32, tag="ibb")
                    nc.sync.dma_start(
                        out=ibb[:],
                        in_=IBR[0:1, t0 * P:(t0 + cfg.b_tiles) * P].to_broadcast(
                            (P, cfg.b_tiles * P)))
                    acc = accpool.tile([P, 136], F32, tag="acc")
                    for ti in range(cfg.b_tiles):
                        T = t0 + ti
                        kvg = kvpool.tile([P, 256], BF16, tag="kvg")
                        gi = nc.gpsimd.indirect_dma_start(
                            out=kvg[:], out_offset=None, in_=KVT[:],
                            in_offset=bass.IndirectOffsetOnAxis(
                                ap=src_sb[:, T:T + 1], axis=0))
                        if mq > 1:
                            q = T % mq
                            if q:
                                gi.ins.queue = f"qPoolDynamic{q}"
                        qx = qxpool.tile([P, 144], F32, tag="qx")
                        oh2 = wpool.tile([P, P], BF16, tag="oh2")
                        nc.vector.tensor_scalar(
                            out=oh2[:], in0=ibb[:, ti * P:(ti + 1) * P],
                            scalar1=iotac_f[:, 0:1], scalar2=None, op0=AX.is_equal)
                        nc.tensor.matmul(out=qx[:, 0:128], lhsT=oh2[:], rhs=qwin,
                                         start=True, stop=True)
                        ttile = wpool.tile([P, P], BF16, tag="ttile")
                        nc.vector.tensor_tensor(
                            out=ttile[:], in0=kvg[:, 0:128], in1=qx[:, 0:128],
                            op=AX.mult)
                        ttp = ttpool.tile([P, P], BF16, tag="ttp")
                        nc.tensor.transpose(out=ttp[:], in_=ttile[:],
                                            identity=ident[:])
                        tts = wpool.tile([P, P], BF16, tag="tts")
                        nc.scalar.activation(out=tts[:], in_=ttp[:], func=AF.Copy)
                        nc.tensor.matmul(out=qx[:, 128:144], lhsT=tts[:],
                                         rhs=wblk[:], start=True, stop=True)
                        u = wpool.tile([P, 8], F32, tag="u")
                        nc.vector.tensor_scalar(
                            out=u[:], in0=qx[:, 128:136],
                            scalar1=a_sb[:, T:T + 1], scalar2=None, op0=AX.mult)
                        sc = wpool.tile([P, 8], F32, tag="sc")
                        nc.vector.tensor_tensor(out=sc[:], in0=u[:],
                                                in1=qx[:, 136:144], op=AX.add)
                        scc = wpool.tile([P, 8], F32, tag="scc")
                        nc.vector.tensor_scalar(out=scc[:], in0=sc[:],
                                                scalar1=5.0, scalar2=-5.0,
                                                op0=AX.min, op1=AX.max)
                        msgz = wpool.tile([P, 136], BF16, tag="msgz")
                        nc.scalar.activation(out=msgz[:, 128:136], in_=scc[:],
                                             func=AF.Exp)
                        nc.vector.tensor_tensor(
                            out=msgz[:, 0:128].rearrange("p (h d) -> p h d", d=16),
                            in0=kvg[:, 128:256].rearrange("p (h d) -> p h d", d=16),
                            in1=msgz[:, 128:136][:, :, None].to_broadcast((P, 8, 16)),
                            op=AX.mult)
                        oh = wpool.tile([P, P], BF16, tag="oh")
                        nc.vector.tensor_scalar(
                            out=oh[:], in0=iota_f[:],
                            scalar1=ib_sb[:, T:T + 1], scalar2=None,
                            op0=AX.is_equal)
                        nc.tensor.matmul(out=acc[:], lhsT=oh[:], rhs=msgz[:],
                                         start=(ti == 0),
                                         stop=(ti == cfg.b_tiles - 1))
                    zp = wpool.tile([P, 8], F32, tag="zp")
                    nc.vector.tensor_scalar(out=zp[:], in0=acc[:, 128:136],
                                            scalar1=1e-6, scalar2=None, op0=AX.add)
                    rt = wpool.tile([P, 8], F32, tag="rt")
                    nc.vector.reciprocal(out=rt[:], in_=zp[:])
                    ho = wpool.tile([P, P], F32, tag="ho")
                    nc.vector.tensor_tensor(
                        out=ho[:].rearrange("p (h d) -> p h d", d=16),
                        in0=acc[:, 0:128].rearrange("p (h d) -> p h d", d=16),
                        in1=rt[:][:, :, None].to_broadcast((P, 8, 16)),
                        op=AX.mult)
                    nc.sync.dma_start(out=HOUT[blk * P:(blk + 1) * P, :], in_=ho[:])
    return nc


def host_prep(x, eidx, eattr, cfg: Cfg):
    """Index-only edge prep + dtype/layout prep of inputs. Returns in_maps."""
    n, _ = x.shape
    src = np.asarray(eidx[0], dtype=np.int64)
    dst = np.asarray(eidx[1], dtype=np.int64)
    a = np.asarray(eattr, dtype=np.float32).reshape(-1)

    order = np.argsort(dst, kind="stable")
    src_s, dst_s, a_s = src[order], dst[order], a[order]
    core_of = np.minimum(dst_s // cfg.npc, cfg.n_cores - 1)

    per_core = []
    maxcnt = 0
    for c in range(cfg.n_cores):
        m = core_of == c
        s, d, av = src_s[m], dst_s[m] - c * cfg.npc, a_s[m]
        blk = d >> 7
        cnt = np.bincount(blk, minlength=cfg.nb)
        maxcnt = max(maxcnt, int(cnt.max()) if len(cnt) else 0)
        per_core.append((s, d, av, blk, cnt))
    cfg.b_tiles = math.ceil(max(maxcnt, 1) / P)
    cfg.tt = cfg.nb * cfg.b_tiles

    xt_pad = np.zeros((cfg.npad, x.shape[1]), dtype=np.float32)
    xt_pad[:n] = x
    xt_bf = np.ascontiguousarray(xt_pad.T).astype(ml_dtypes.bfloat16)

    in_maps = []
    for c in range(cfg.n_cores):
        s, d, av, blk, cnt = per_core[c]
        starts = np.zeros(cfg.nb, dtype=np.int64)
        starts[1:] = np.cumsum(cnt)[:-1]
        pos = np.arange(len(s)) - starts[blk]
        slot = blk * (cfg.b_tiles * P) + pos

        tot = cfg.tt * P
        SRCa = np.zeros(tot, dtype=np.int32)
        Aa = np.zeros(tot, dtype=np.float32)
        IBa = np.full(tot, 200.0, dtype=np.float32)
        SRCa[slot] = s
        Aa[slot] = av
        IBa[slot] = (d - blk * P).astype(np.float32)

        def tmat(arr):
            return np.ascontiguousarray(arr.reshape(cfg.tt, P).T)

        in_maps.append({
            "xt": xt_bf,
            "xtq": np.ascontiguousarray(
                xt_bf[:, c * cfg.npc:c * cfg.npc + cfg.qrows]),
            "srct": tmat(SRCa),
            "at": tmat(Aa),
            "ibt": tmat(IBa),
            "ibr": np.ascontiguousarray(IBa.reshape(1, cfg.tt * P)).astype(np.float32),
        })
    return in_maps


def _weight_maps(Wq, bq, Wk, bk, We, be, Wv, bv):
    wkv = np.concatenate([Wk, Wv], axis=1).astype(np.float32)
    return {
        "wkv": np.ascontiguousarray(wkv),
        "bkv": np.concatenate([bk, bv]).astype(np.float32).reshape(1, 256),
        "wq": np.ascontiguousarray(Wq.astype(np.float32)),
        "bq": bq.astype(np.float32).reshape(1, -1),
        "we": We.astype(np.float32).reshape(1, -1),
        "be": be.astype(np.float32).reshape(1, -1),
    }


def kernel(**inputs):
    global LAST_RESULTS
    _install_wait_fix()
    x = np.asarray(inputs["x"], dtype=np.float32)
    cfg = Cfg(n_nodes=x.shape[0])
    in_maps = host_prep(x, inputs["expander_edge_index"],
                        inputs["expander_edge_attr"], cfg)
    wm = _weight_maps(np.asarray(inputs["Wq"]), np.asarray(inputs["bq"]),
                      np.asarray(inputs["Wk"]), np.asarray(inputs["bk"]),
                      np.asarray(inputs["We"]), np.asarray(inputs["be"]),
                      np.asarray(inputs["Wv"]), np.asarray(inputs["bv"]))
    for im in in_maps:
        im.update(wm)

    nc = build_nc(cfg)
    trace = os.environ.get("KERNEL_TRACE", "0") == "1"
    res = run_bass_kernel_spmd(nc, in_maps, list(range(cfg.n_cores)), trace=trace)
    LAST_RESULTS = res

    out = np.empty((x.shape[0], x.shape[1]), dtype=np.float32)
    for c in range(cfg.n_cores):
        out[c * cfg.npc:(c + 1) * cfg.npc] = res.results[c]["hout"][:cfg.npc]
    return out

